# revision 1
# baseline (speedup 1.0000x reference)
"""Trainium2 Bass kernel for nn_Classifier (segment mean-pool + tiny MLP head).

Pipeline (matches the jax reference):
  pooled[g] = mean of features over nodes with batch id g   (2048 graphs)
  out = LeakyReLU(LayerNorm(pooled @ W1 + b1)) @ W2 + b2    -> [2048, 1]

Sharding strategy: the batch ids are sorted, so nodes are split across the 8
cores at segment-block boundaries — core i owns graphs [256i, 256i+256) and
exactly the nodes belonging to them. Each core computes segment sums for its
own 256 graphs (disjoint), so no collective is needed; the host concatenates
the 8 per-core [256]-sized outputs.

Per core, segment sums are computed on the tensor engine: for each 128-node
sub-tile, a one-hot matrix [128 nodes, 128 segs] is built on the vector engine
(iota-vs-segment-id compare) and used as the matmul stationary against the
node features [128, 257] (a ones column is appended to also accumulate the
per-segment counts), accumulating into PSUM. Features stream in ~1 MB DMA
chunks of 8 sub-tiles for good HBM efficiency.
"""

from contextlib import ExitStack

import numpy as np

import concourse.bass as bass
import concourse.mybir as mybir
import concourse.tile as tile
from concourse.bass_utils import run_bass_kernel_spmd

# ---------------------------------------------------------------------------
# Workaround: this walrus build rejects instructions carrying more than one
# semaphore wait ("Too many sync wait commands"), but Tile's semaphore
# assignment freely attaches several. After the TileContext has lowered the
# program, split any excess waits onto same-engine nops inserted right before
# the instruction (semantics are identical: all waits are monotonic and must
# hold before the instruction issues).
_MAX_WAITS = 1


def _split_excess_waits(nc: "bass.Bass", max_waits: int = _MAX_WAITS) -> None:
    ctr = 0
    for f in nc.m.functions:
        for b in f.blocks:
            out = []
            for inst in b.instructions:
                si = inst.sync_info
                waits = list(si.on_wait) if (si is not None and si.on_wait) else []
                if len(waits) > max_waits:
                    keep = waits[-max_waits:]
                    extra = waits[:-max_waits]
                    # On the PE queue the carrier must be a DRAIN: silicon
                    # promotes waitless LDWEIGHTS past in-flight work, so a
                    # plain nop's wait can be bypassed (walrus attaches a
                    # matmul's waits to its LDWEIGHTS — stripping them onto a
                    # nop re-opens that race). A drain fully serializes.
                    is_pe = inst.engine == mybir.EngineType.PE
                    for i in range(0, len(extra), max_waits):
                        ctr += 1
                        if is_pe:
                            nop = mybir.InstDrain(
                                name=f"waitsplit_drain_{ctr}", ins=[], outs=[],
                                engine=inst.engine,
                            )
                        else:
                            nop = mybir.InstNoOp(
                                name=f"waitsplit_nop_{ctr}", ins=[], outs=[],
                                engine=inst.engine,
                            )
                        nop.sync_info = mybir.SyncInfo(
                            on_wait=extra[i : i + max_waits], on_update=[]
                        )
                        nc.register_instruction(nop)
                        out.append(nop)
                    inst.sync_info = mybir.SyncInfo(
                        on_wait=keep, on_update=list(si.on_update or [])
                    )
                out.append(inst)
            b.instructions = out
# ---------------------------------------------------------------------------

N_CORES = 8
NUM_GRAPHS = 2048
SEGS_PER_CORE = NUM_GRAPHS // N_CORES  # 256
N_BLOCKS = NUM_GRAPHS // 128  # 16 blocks of 128 segments; 2 per core
D = 256
DA = D + 2  # features + ones column (for counts) + pad (fp32r needs even N)
K_SUB = 8  # 128-node sub-tiles per DMA chunk (chunk = 1024 nodes, ~1 MB)
CHUNK = 128 * K_SUB
LN_EPS = 1e-5
NEG_SLOPE = 0.01

_F32 = mybir.dt.float32
_F32R = mybir.dt.float32r
_ALU = mybir.AluOpType

# Test/debug hooks: set PROFILE=True before calling kernel() to request an
# NTFF trace; the BassKernelResults lands in LAST_RESULT.
PROFILE = False
PROFILE_DIR = None
LAST_RESULT = None


def _build_program(chunks_per_region: int) -> bass.Bass:
    R = chunks_per_region
    C = 2 * R  # chunks per core (2 segment blocks of 128)
    n_nodes = C * CHUNK

    nc = bass.Bass("TRN2", debug=False)
    # float32r: same bits as fp32, but the PE consumes it on the replicated
    # datapath at 1 cycle/row (vs 4 for fp32), rounding inputs to ~12 mantissa
    # bits. Segment sums tolerate this (~1e-4 scale-relative).
    feat = nc.dram_tensor("feat", [n_nodes, DA], _F32R, kind="ExternalInput").ap()
    segT = nc.dram_tensor("segT", [128, C * K_SUB], _F32, kind="ExternalInput").ap()
    iota_d = nc.dram_tensor("iota", [128, 128], _F32, kind="ExternalInput").ap()
    ident_d = nc.dram_tensor("ident", [128, 128], _F32, kind="ExternalInput").ap()
    w1aug_d = nc.dram_tensor("w1aug", [D + 1, 128], _F32, kind="ExternalInput").ap()
    pvec_d = nc.dram_tensor("pvec", [1, 385], _F32, kind="ExternalInput").ap()
    out_d = nc.dram_tensor("out", [2, 128], _F32, kind="ExternalOutput").ap()

    with tile.TileContext(nc) as tc, ExitStack() as ctx:
        cpool = ctx.enter_context(tc.tile_pool(name="consts", bufs=1))
        fpool = ctx.enter_context(tc.tile_pool(name="feat", bufs=8))
        opool = ctx.enter_context(tc.tile_pool(name="oh", bufs=12))
        acc = ctx.enter_context(tc.tile_pool(name="acc", bufs=1, space="PSUM"))
        ppool = ctx.enter_context(tc.tile_pool(name="pw", bufs=2, space="PSUM"))
        spool = ctx.enter_context(tc.tile_pool(name="small", bufs=2))

        iota_t = cpool.tile([128, 128], _F32, tag="iota")
        nc.sync.dma_start(out=iota_t[:], in_=iota_d[:])
        ident_t = cpool.tile([128, 128], _F32, tag="ident")
        nc.sync.dma_start(out=ident_t[:], in_=ident_d[:])
        segT_t = cpool.tile([128, C * K_SUB], _F32, tag="segT")
        nc.sync.dma_start(out=segT_t[:], in_=segT[:])
        w1a = cpool.tile([128, 128], _F32, tag="w1a")
        nc.sync.dma_start(out=w1a[:], in_=w1aug_d[0:128, :])
        w1b = cpool.tile([128, 128], _F32, tag="w1b")
        nc.sync.dma_start(out=w1b[:], in_=w1aug_d[128:256, :])
        w1c = cpool.tile([1, 128], _F32, tag="w1c")
        nc.sync.dma_start(out=w1c[:], in_=w1aug_d[256:257, :])
        pv = cpool.tile([1, 385], _F32, tag="pv")
        nc.sync.dma_start(out=pv[:], in_=pvec_d[:])
        ones_row = cpool.tile([1, 256], _F32, tag="ones")
        nc.vector.memset(ones_row[:], 1.0)
        epsc = cpool.tile([128, 1], _F32, tag="epsc")
        nc.vector.memset(epsc[:], LN_EPS)

        # broadcast [gamma | beta | W2 | b2] to all 128 partitions
        bc_ps = ppool.tile([128, 385], _F32, tag="bc")
        nc.tensor.matmul(
            out=bc_ps[:], lhsT=ones_row[:, 0:128], rhs=pv[:], start=True, stop=True
        )
        bc = cpool.tile([128, 385], _F32, tag="bcs")
        nc.scalar.copy(bc[:], bc_ps[:])

        # ---- main stream: per-segment sums (and counts in column 256) ----
        sums = [acc.tile([128, DA], _F32, tag=f"sum{r}", name=f"sum{r}") for r in range(2)]
        for r in range(2):
            for c in range(R):
                chunk = r * R + c
                ft = fpool.tile([128, K_SUB, DA], _F32R, tag="ft")
                src = feat[chunk * CHUNK : (chunk + 1) * CHUNK, :].rearrange(
                    "(p k) f -> p k f", p=128
                )
                dma_eng = nc.sync if chunk % 2 == 0 else nc.scalar
                dma_eng.dma_start(out=ft[:], in_=src)
                for k in range(K_SUB):
                    col = chunk * K_SUB + k
                    oh = opool.tile([128, 128], _F32R, tag="oh")
                    oh_eng = nc.vector if k % 2 == 0 else nc.gpsimd
                    oh_eng.tensor_scalar(
                        out=oh[:],
                        in0=iota_t[:],
                        scalar1=segT_t[:, col : col + 1],
                        scalar2=None,
                        op0=_ALU.is_equal,
                    )
                    # float32r: exact fp32 matmul on the replicated datapath —
                    # 1 cycle/row at moving free dim >= 256 vs fp32's 4.
                    nc.tensor.matmul(
                        out=sums[r][:],
                        lhsT=oh[:],
                        rhs=ft[:, k, :],
                        start=(c == 0 and k == 0),
                        stop=(c == R - 1 and k == K_SUB - 1),
                    )

        # ---- pooled = sums / max(counts, 1), transposed for the head ----
        ptT = [spool.tile([128, 256], _F32, tag=f"ptT{fb}", name=f"ptT{fb}") for fb in range(2)]
        for r in range(2):
            cnt = spool.tile([128, 1], _F32, tag="cnt")
            nc.vector.tensor_scalar(
                out=cnt[:], in0=sums[r][:, 256:257], scalar1=1.0, scalar2=None,
                op0=_ALU.max,
            )
            rec = spool.tile([128, 1], _F32, tag="rec")
            nc.vector.reciprocal(rec[:], cnt[:])
            pooled = spool.tile([128, 256], _F32, tag="pooled")
            nc.vector.tensor_scalar(
                out=pooled[:], in0=sums[r][:, 0:256], scalar1=rec[:], scalar2=None,
                op0=_ALU.mult,
            )
            for fb in range(2):
                tp = ppool.tile([128, 128], _F32, tag="tp")
                nc.tensor.transpose(
                    out=tp[:], in_=pooled[:, fb * 128 : (fb + 1) * 128],
                    identity=ident_t[:],
                )
                nc.scalar.copy(ptT[fb][:, r * 128 : (r + 1) * 128], tp[:])

        # ---- head: h = pooled @ W1 + b1; LayerNorm; LeakyReLU; @ W2 + b2 ----
        for m in range(2):
            msl = slice(m * 128, (m + 1) * 128)
            h_ps = ppool.tile([128, 128], _F32, tag="h")
            nc.tensor.matmul(
                out=h_ps[:], lhsT=ptT[0][:, msl], rhs=w1a[:], start=True, stop=False
            )
            nc.tensor.matmul(
                out=h_ps[:], lhsT=ptT[1][:, msl], rhs=w1b[:], start=False, stop=False
            )
            nc.tensor.matmul(
                out=h_ps[:], lhsT=ones_row[:, msl], rhs=w1c[:], start=False, stop=True
            )

            musum = spool.tile([128, 1], _F32, tag="musum")
            nc.vector.tensor_reduce(
                out=musum[:], in_=h_ps[:], axis=mybir.AxisListType.X, op=_ALU.add
            )
            mu = spool.tile([128, 1], _F32, tag="mu")
            nc.vector.tensor_scalar(
                out=mu[:], in0=musum[:], scalar1=1.0 / 128, scalar2=None, op0=_ALU.mult
            )
            hc = spool.tile([128, 128], _F32, tag="hc")
            nc.vector.tensor_scalar(
                out=hc[:], in0=h_ps[:], scalar1=mu[:], scalar2=None, op0=_ALU.subtract
            )
            sq = spool.tile([128, 128], _F32, tag="sq")
            ssq = spool.tile([128, 1], _F32, tag="ssq")
            nc.vector.scalar_tensor_tensor(
                out=sq[:], in0=hc[:], scalar=1.0, in1=hc[:],
                op0=_ALU.mult, op1=_ALU.mult, accum_out=ssq[:],
            )
            std = spool.tile([128, 1], _F32, tag="std")
            nc.scalar.activation(
                std[:], ssq[:], mybir.ActivationFunctionType.Sqrt,
                bias=epsc[:], scale=1.0 / 128,
            )
            rstd = spool.tile([128, 1], _F32, tag="rstd")
            nc.vector.reciprocal(rstd[:], std[:])
            y = spool.tile([128, 128], _F32, tag="y")
            nc.vector.scalar_tensor_tensor(
                out=y[:], in0=hc[:], scalar=rstd[:], in1=bc[:, 0:128],
                op0=_ALU.mult, op1=_ALU.mult,
            )
            y2 = spool.tile([128, 128], _F32, tag="y2")
            nc.vector.tensor_tensor(out=y2[:], in0=y[:], in1=bc[:, 128:256],
                                    op=_ALU.add)
            yl = spool.tile([128, 128], _F32, tag="yl")
            nc.vector.scalar_tensor_tensor(
                out=yl[:], in0=y2[:], scalar=NEG_SLOPE, in1=y2[:],
                op0=_ALU.mult, op1=_ALU.max,
            )
            prod = spool.tile([128, 128], _F32, tag="prod")
            oc = spool.tile([128, 1], _F32, tag="oc")
            nc.vector.scalar_tensor_tensor(
                out=prod[:], in0=yl[:], scalar=1.0, in1=bc[:, 256:384],
                op0=_ALU.mult, op1=_ALU.mult, accum_out=oc[:],
            )
            ofin = spool.tile([128, 1], _F32, tag="ofin")
            nc.vector.tensor_scalar(
                out=ofin[:], in0=oc[:], scalar1=bc[:, 384:385], scalar2=None,
                op0=_ALU.add,
            )
            nc.sync.dma_start(out=out_d[m, :], in_=ofin[:])

    _split_excess_waits(nc)
    return nc


def _prep_inputs(features, batch):
    """Segment-block-aligned sharding + per-core padded arrays."""
    feats = np.ascontiguousarray(np.asarray(features), dtype=np.float32)
    seg = np.asarray(batch).astype(np.int64)
    n = seg.shape[0]
    counts = np.bincount(seg, minlength=NUM_GRAPHS)
    bnd = np.zeros(NUM_GRAPHS + 1, np.int64)
    bnd[1:] = np.cumsum(counts)

    block_lo = bnd[0 : NUM_GRAPHS : 128]
    block_hi = bnd[128 : NUM_GRAPHS + 1 : 128]
    block_n = block_hi - block_lo  # nodes per 128-segment block (16 blocks)
    R = int(np.max((block_n + CHUNK - 1) // CHUNK))  # chunks per region
    region = R * CHUNK
    ncap = 2 * region

    feat_aug = np.zeros((N_CORES, ncap, DA), np.float32)
    seg_adj = np.full((N_CORES, ncap), -1.0, np.float32)
    for i in range(N_CORES):
        for r in range(2):
            b = 2 * i + r
            lo, hi = int(block_lo[b]), int(block_hi[b])
            m = hi - lo
            off = r * region
            feat_aug[i, off : off + m, :D] = feats[lo:hi]
            feat_aug[i, off : off + m, D] = 1.0
            seg_adj[i, off : off + m] = (seg[lo:hi] - 128 * b).astype(np.float32)
    # transpose seg ids to match the on-chip [partition, sub-tile] layout:
    # node (chunk*1024 + p*8 + k) -> segT[p, chunk*8 + k]
    segT = (
        seg_adj.reshape(N_CORES, -1, 128, K_SUB)
        .transpose(0, 2, 1, 3)
        .reshape(N_CORES, 128, -1)
    )
    return feat_aug, np.ascontiguousarray(segT), R


def kernel(features, batch, W1, b1, gamma, beta, W2, b2):
    feat_aug, segT, R = _prep_inputs(features, batch)

    iota = np.tile(np.arange(128, dtype=np.float32)[None, :], (128, 1))
    ident = np.eye(128, dtype=np.float32)
    w1aug = np.concatenate(
        [np.asarray(W1, np.float32), np.asarray(b1, np.float32)[None, :]], axis=0
    )
    pvec = np.concatenate(
        [
            np.asarray(gamma, np.float32).ravel(),
            np.asarray(beta, np.float32).ravel(),
            np.asarray(W2, np.float32).ravel(),
            np.asarray(b2, np.float32).ravel(),
        ]
    )[None, :]

    nc = _build_program(R)
    in_maps = [
        {
            "feat": feat_aug[i],
            "segT": segT[i],
            "iota": iota,
            "ident": ident,
            "w1aug": w1aug,
            "pvec": pvec,
        }
        for i in range(N_CORES)
    ]
    res = run_bass_kernel_spmd(
        nc, in_maps, list(range(N_CORES)), trace=PROFILE, tmpdir=PROFILE_DIR
    )
    global LAST_RESULT
    LAST_RESULT = res
    out = np.concatenate(
        [res.results[i]["out"].reshape(SEGS_PER_CORE) for i in range(N_CORES)]
    )
    return out.reshape(NUM_GRAPHS, 1).astype(np.float32)



# revision 2
# speedup vs baseline: 5.3888x; 5.3888x over previous
"""Trainium2 Bass kernel for nn_Classifier (segment mean-pool + tiny MLP head).

Pipeline (matches the jax reference):
  pooled[g] = mean of features over nodes with batch id g   (2048 graphs)
  out = LeakyReLU(LayerNorm(pooled @ W1 + b1)) @ W2 + b2    -> [2048, 1]

Sharding: batch ids are sorted, so nodes split across the 8 cores at
segment-block boundaries — core i owns graphs [256i, 256i+256) and exactly the
nodes belonging to them. Each core computes segment sums for its own 256
graphs (disjoint), so no collective is needed; the host concatenates the 8
per-core [256]-sized outputs.

v2 changes vs the original baseline (measured 670 us on this box):
 - features stream in bf16 (rel err ~2e-3 end to end, 10x under the gate),
   halving HBM traffic: ~32.5 MB/core -> ~91 us at ~358 GB/s.
 - the one-hot matrices are built in bf16 (the old fp32r destination put
   every DVE/GPSIMD tensor_scalar on a ~1.5-2.6 us slow path), and in batches
   of 16 subtiles: one scalar-engine broadcast-copy of the segment ids plus
   one vector-engine tensor_tensor is_equal per 2 chunks, instead of 512
   per-subtile per-partition-scalar ops.
 - GPSIMD is not used at all (its tensor_scalar measured 2.6 us/op).
 - segment counts come from host bincount (reciprocals are an input), so the
   ones-column (and the fp32r padding) is gone: matmul free dim is exactly 256.
"""

from contextlib import ExitStack

import numpy as np
import ml_dtypes

import concourse.bass as bass
import concourse.mybir as mybir
import concourse.tile as tile
from concourse.bass_utils import run_bass_kernel_spmd

# ---------------------------------------------------------------------------
# Workaround: this walrus build rejects instructions carrying more than one
# semaphore wait ("Too many sync wait commands"), but Tile's semaphore
# assignment freely attaches several. After the TileContext has lowered the
# program, split any excess waits onto same-engine nops inserted right before
# the instruction (semantics are identical: all waits are monotonic and must
# hold before the instruction issues).
_MAX_WAITS = 1


def _split_excess_waits(nc: "bass.Bass", max_waits: int = _MAX_WAITS) -> None:
    ctr = 0
    for f in nc.m.functions:
        for b in f.blocks:
            out = []
            for inst in b.instructions:
                si = inst.sync_info
                waits = list(si.on_wait) if (si is not None and si.on_wait) else []
                if len(waits) > max_waits:
                    keep = waits[-max_waits:]
                    extra = waits[:-max_waits]
                    # On the PE queue the carrier must be a DRAIN: silicon
                    # promotes waitless LDWEIGHTS past in-flight work, so a
                    # plain nop's wait can be bypassed (walrus attaches a
                    # matmul's waits to its LDWEIGHTS — stripping them onto a
                    # nop re-opens that race). A drain fully serializes.
                    is_pe = inst.engine == mybir.EngineType.PE
                    for i in range(0, len(extra), max_waits):
                        ctr += 1
                        if is_pe:
                            nop = mybir.InstDrain(
                                name=f"waitsplit_drain_{ctr}", ins=[], outs=[],
                                engine=inst.engine,
                            )
                        else:
                            nop = mybir.InstNoOp(
                                name=f"waitsplit_nop_{ctr}", ins=[], outs=[],
                                engine=inst.engine,
                            )
                        nop.sync_info = mybir.SyncInfo(
                            on_wait=extra[i : i + max_waits], on_update=[]
                        )
                        nc.register_instruction(nop)
                        out.append(nop)
                    inst.sync_info = mybir.SyncInfo(
                        on_wait=keep, on_update=list(si.on_update or [])
                    )
                out.append(inst)
            b.instructions = out
# ---------------------------------------------------------------------------

N_CORES = 8
NUM_GRAPHS = 2048
SEGS_PER_CORE = NUM_GRAPHS // N_CORES  # 256
D = 256
K_SUB = 8  # 128-node sub-tiles per DMA chunk (chunk = 1024 nodes, 512 KB bf16)
CHUNK = 128 * K_SUB
OH_CHUNKS = 2  # chunks per one-hot generation batch (16 subtiles, 2048 cols)
LN_EPS = 1e-5
NEG_SLOPE = 0.01

_F32 = mybir.dt.float32
_BF16 = mybir.dt.bfloat16
_ALU = mybir.AluOpType

# Test/debug hooks: set PROFILE=True before calling kernel() to request an
# NTFF trace; the BassKernelResults lands in LAST_RESULT.
PROFILE = False
PROFILE_DIR = None
LAST_RESULT = None


def _build_program(chunks_per_region: int) -> bass.Bass:
    R = chunks_per_region
    C = 2 * R  # chunks per core (2 segment blocks of 128)
    n_nodes = C * CHUNK
    OH_COLS = OH_CHUNKS * K_SUB  # one-hot batch: 16 subtile columns

    nc = bass.Bass("TRN2", debug=False)
    feat = nc.dram_tensor("feat", [n_nodes, D], _BF16, kind="ExternalInput").ap()
    segT = nc.dram_tensor("segT", [128, C * K_SUB], _BF16, kind="ExternalInput").ap()
    iotab_d = nc.dram_tensor("iotab", [128, OH_COLS * 128], _BF16, kind="ExternalInput").ap()
    ident_d = nc.dram_tensor("ident", [128, 128], _F32, kind="ExternalInput").ap()
    w1aug_d = nc.dram_tensor("w1aug", [D + 1, 128], _F32, kind="ExternalInput").ap()
    pvec_d = nc.dram_tensor("pvec", [1, 385], _F32, kind="ExternalInput").ap()
    recip_d = nc.dram_tensor("recip", [128, 2], _F32, kind="ExternalInput").ap()
    out_d = nc.dram_tensor("out", [2, 128], _F32, kind="ExternalOutput").ap()

    with tile.TileContext(nc) as tc, ExitStack() as ctx:
        cpool = ctx.enter_context(tc.tile_pool(name="consts", bufs=1))
        fpool = ctx.enter_context(tc.tile_pool(name="feat", bufs=6))
        opool = ctx.enter_context(tc.tile_pool(name="oh", bufs=3))
        acc = ctx.enter_context(tc.tile_pool(name="acc", bufs=1, space="PSUM"))
        ppool = ctx.enter_context(tc.tile_pool(name="pw", bufs=2, space="PSUM"))
        spool = ctx.enter_context(tc.tile_pool(name="small", bufs=2))

        iotab_t = cpool.tile([128, OH_COLS, 128], _BF16, tag="iotab")
        nc.sync.dma_start(out=iotab_t[:], in_=iotab_d[:].rearrange("p (c s) -> p c s", s=128))
        ident_t = cpool.tile([128, 128], _F32, tag="ident")
        nc.sync.dma_start(out=ident_t[:], in_=ident_d[:])
        segT_t = cpool.tile([128, C * K_SUB], _BF16, tag="segT")
        nc.sync.dma_start(out=segT_t[:], in_=segT[:])
        w1a = cpool.tile([128, 128], _F32, tag="w1a")
        nc.sync.dma_start(out=w1a[:], in_=w1aug_d[0:128, :])
        w1b = cpool.tile([128, 128], _F32, tag="w1b")
        nc.sync.dma_start(out=w1b[:], in_=w1aug_d[128:256, :])
        w1c = cpool.tile([1, 128], _F32, tag="w1c")
        nc.sync.dma_start(out=w1c[:], in_=w1aug_d[256:257, :])
        pv = cpool.tile([1, 385], _F32, tag="pv")
        nc.sync.dma_start(out=pv[:], in_=pvec_d[:])
        recip_t = cpool.tile([128, 2], _F32, tag="recip")
        nc.sync.dma_start(out=recip_t[:], in_=recip_d[:])
        ones_row = cpool.tile([1, 256], _F32, tag="ones")
        nc.vector.memset(ones_row[:], 1.0)
        epsc = cpool.tile([128, 1], _F32, tag="epsc")
        nc.vector.memset(epsc[:], LN_EPS)

        # broadcast [gamma | beta | W2 | b2] to all 128 partitions
        bc_ps = ppool.tile([128, 385], _F32, tag="bc")
        nc.tensor.matmul(
            out=bc_ps[:], lhsT=ones_row[:, 0:128], rhs=pv[:], start=True, stop=True
        )
        bc = cpool.tile([128, 385], _F32, tag="bcs")
        nc.scalar.copy(bc[:], bc_ps[:])

        # ---- main stream: per-segment sums via bf16 one-hot matmuls ----
        sums = [acc.tile([128, D], _F32, tag=f"sum{r}", name=f"sum{r}") for r in range(2)]
        oh = None
        for r in range(2):
            for c in range(R):
                chunk = r * R + c
                ft = fpool.tile([128, K_SUB, D], _BF16, tag="ft")
                src = feat[chunk * CHUNK : (chunk + 1) * CHUNK, :].rearrange(
                    "(p k) f -> p k f", p=128
                )
                dma_eng = nc.sync if chunk % 2 == 0 else nc.scalar
                dma_eng.dma_start(out=ft[:], in_=src)
                if chunk % OH_CHUNKS == 0:
                    c0 = chunk * K_SUB
                    segB = opool.tile([128, OH_COLS, 128], _BF16, tag="segB")
                    nc.scalar.copy(
                        segB[:],
                        segT_t[:, c0 : c0 + OH_COLS]
                        .unsqueeze(2)
                        .broadcast_to([128, OH_COLS, 128]),
                    )
                    oh = opool.tile([128, OH_COLS, 128], _BF16, tag="oh")
                    nc.vector.tensor_tensor(
                        out=oh[:], in0=segB[:], in1=iotab_t[:], op=_ALU.is_equal
                    )
                for k in range(K_SUB):
                    j = (chunk % OH_CHUNKS) * K_SUB + k
                    nc.tensor.matmul(
                        out=sums[r][:],
                        lhsT=oh[:, j, :],
                        rhs=ft[:, k, :],
                        start=(c == 0 and k == 0),
                        stop=(c == R - 1 and k == K_SUB - 1),
                    )

        # ---- pooled = sums * recip(counts), transposed for the head ----
        ptT = [spool.tile([128, 256], _F32, tag=f"ptT{fb}", name=f"ptT{fb}") for fb in range(2)]
        for r in range(2):
            pooled = spool.tile([128, 256], _F32, tag="pooled")
            nc.vector.tensor_scalar(
                out=pooled[:], in0=sums[r][:], scalar1=recip_t[:, r : r + 1],
                scalar2=None, op0=_ALU.mult,
            )
            for fb in range(2):
                tp = ppool.tile([128, 128], _F32, tag="tp")
                nc.tensor.transpose(
                    out=tp[:], in_=pooled[:, fb * 128 : (fb + 1) * 128],
                    identity=ident_t[:],
                )
                nc.scalar.copy(ptT[fb][:, r * 128 : (r + 1) * 128], tp[:])

        # ---- head: h = pooled @ W1 + b1; LayerNorm; LeakyReLU; @ W2 + b2 ----
        for m in range(2):
            msl = slice(m * 128, (m + 1) * 128)
            h_ps = ppool.tile([128, 128], _F32, tag="h")
            nc.tensor.matmul(
                out=h_ps[:], lhsT=ptT[0][:, msl], rhs=w1a[:], start=True, stop=False
            )
            nc.tensor.matmul(
                out=h_ps[:], lhsT=ptT[1][:, msl], rhs=w1b[:], start=False, stop=False
            )
            nc.tensor.matmul(
                out=h_ps[:], lhsT=ones_row[:, msl], rhs=w1c[:], start=False, stop=True
            )

            musum = spool.tile([128, 1], _F32, tag="musum")
            nc.vector.tensor_reduce(
                out=musum[:], in_=h_ps[:], axis=mybir.AxisListType.X, op=_ALU.add
            )
            mu = spool.tile([128, 1], _F32, tag="mu")
            nc.vector.tensor_scalar(
                out=mu[:], in0=musum[:], scalar1=1.0 / 128, scalar2=None, op0=_ALU.mult
            )
            hc = spool.tile([128, 128], _F32, tag="hc")
            nc.vector.tensor_scalar(
                out=hc[:], in0=h_ps[:], scalar1=mu[:], scalar2=None, op0=_ALU.subtract
            )
            sq = spool.tile([128, 128], _F32, tag="sq")
            ssq = spool.tile([128, 1], _F32, tag="ssq")
            nc.vector.scalar_tensor_tensor(
                out=sq[:], in0=hc[:], scalar=1.0, in1=hc[:],
                op0=_ALU.mult, op1=_ALU.mult, accum_out=ssq[:],
            )
            std = spool.tile([128, 1], _F32, tag="std")
            nc.scalar.activation(
                std[:], ssq[:], mybir.ActivationFunctionType.Sqrt,
                bias=epsc[:], scale=1.0 / 128,
            )
            rstd = spool.tile([128, 1], _F32, tag="rstd")
            nc.vector.reciprocal(rstd[:], std[:])
            y = spool.tile([128, 128], _F32, tag="y")
            nc.vector.scalar_tensor_tensor(
                out=y[:], in0=hc[:], scalar=rstd[:], in1=bc[:, 0:128],
                op0=_ALU.mult, op1=_ALU.mult,
            )
            y2 = spool.tile([128, 128], _F32, tag="y2")
            nc.vector.tensor_tensor(out=y2[:], in0=y[:], in1=bc[:, 128:256],
                                    op=_ALU.add)
            yl = spool.tile([128, 128], _F32, tag="yl")
            nc.vector.scalar_tensor_tensor(
                out=yl[:], in0=y2[:], scalar=NEG_SLOPE, in1=y2[:],
                op0=_ALU.mult, op1=_ALU.max,
            )
            prod = spool.tile([128, 128], _F32, tag="prod")
            oc = spool.tile([128, 1], _F32, tag="oc")
            nc.vector.scalar_tensor_tensor(
                out=prod[:], in0=yl[:], scalar=1.0, in1=bc[:, 256:384],
                op0=_ALU.mult, op1=_ALU.mult, accum_out=oc[:],
            )
            ofin = spool.tile([128, 1], _F32, tag="ofin")
            nc.vector.tensor_scalar(
                out=ofin[:], in0=oc[:], scalar1=bc[:, 384:385], scalar2=None,
                op0=_ALU.add,
            )
            nc.sync.dma_start(out=out_d[m, :], in_=ofin[:])

    _split_excess_waits(nc)
    return nc


def _prep_inputs(features, batch):
    """Segment-block-aligned sharding + per-core padded bf16 arrays."""
    feats = np.asarray(features, dtype=np.float32)
    seg = np.asarray(batch).astype(np.int64)
    counts = np.bincount(seg, minlength=NUM_GRAPHS)
    bnd = np.zeros(NUM_GRAPHS + 1, np.int64)
    bnd[1:] = np.cumsum(counts)

    block_lo = bnd[0 : NUM_GRAPHS : 128]
    block_hi = bnd[128 : NUM_GRAPHS + 1 : 128]
    block_n = block_hi - block_lo  # nodes per 128-segment block (16 blocks)
    R = int(np.max((block_n + CHUNK - 1) // CHUNK))  # chunks per region
    if R % OH_CHUNKS:
        R += OH_CHUNKS - (R % OH_CHUNKS)
    region = R * CHUNK
    ncap = 2 * region

    feat_aug = np.zeros((N_CORES, ncap, D), ml_dtypes.bfloat16)
    seg_adj = np.full((N_CORES, ncap), -1.0, np.float32)
    for i in range(N_CORES):
        for r in range(2):
            b = 2 * i + r
            lo, hi = int(block_lo[b]), int(block_hi[b])
            m = hi - lo
            off = r * region
            feat_aug[i, off : off + m, :] = feats[lo:hi].astype(ml_dtypes.bfloat16)
            seg_adj[i, off : off + m] = (seg[lo:hi] - 128 * b).astype(np.float32)
    # transpose seg ids to match the on-chip [partition, sub-tile] layout:
    # node (chunk*1024 + p*8 + k) -> segT[p, chunk*8 + k]
    segT = (
        seg_adj.reshape(N_CORES, -1, 128, K_SUB)
        .transpose(0, 2, 1, 3)
        .reshape(N_CORES, 128, -1)
        .astype(ml_dtypes.bfloat16)
    )
    # reciprocal mean weights per (core, partition=seg-in-block, region)
    recip = (1.0 / np.maximum(counts.astype(np.float64), 1.0)).astype(np.float32)
    recip = recip.reshape(N_CORES, 2, 128).transpose(0, 2, 1)  # [core, 128, 2]
    return feat_aug, np.ascontiguousarray(segT), np.ascontiguousarray(recip), R


def kernel(features, batch, W1, b1, gamma, beta, W2, b2):
    feat_aug, segT, recip, R = _prep_inputs(features, batch)

    iotab = np.tile(
        np.arange(128, dtype=np.float32)[None, None, :], (128, OH_CHUNKS * K_SUB, 1)
    ).reshape(128, -1).astype(ml_dtypes.bfloat16)
    ident = np.eye(128, dtype=np.float32)
    w1aug = np.concatenate(
        [np.asarray(W1, np.float32), np.asarray(b1, np.float32)[None, :]], axis=0
    )
    pvec = np.concatenate(
        [
            np.asarray(gamma, np.float32).ravel(),
            np.asarray(beta, np.float32).ravel(),
            np.asarray(W2, np.float32).ravel(),
            np.asarray(b2, np.float32).ravel(),
        ]
    )[None, :]

    nc = _build_program(R)
    in_maps = [
        {
            "feat": feat_aug[i],
            "segT": segT[i],
            "iotab": iotab,
            "ident": ident,
            "w1aug": w1aug,
            "pvec": pvec,
            "recip": recip[i],
        }
        for i in range(N_CORES)
    ]
    res = run_bass_kernel_spmd(
        nc, in_maps, list(range(N_CORES)), trace=PROFILE, tmpdir=PROFILE_DIR
    )
    global LAST_RESULT
    LAST_RESULT = res
    out = np.concatenate(
        [res.results[i]["out"].reshape(SEGS_PER_CORE) for i in range(N_CORES)]
    )
    return out.reshape(NUM_GRAPHS, 1).astype(np.float32)
